# revision 17
# baseline (speedup 1.0000x reference)
"""Trainium2 Bass kernel for nn_Attention_Encode (dense transformer block).

Reference computation (per batch b):
    ZTU  = ZT[b] @ W.T            (2048, 1024) -> heads (16, 2048, 64)
    S_h  = ZTU_h @ ZTU_h.T * s    (2048, 2048)   symmetric! (Q == K)
    P_h  = softmax(S_h)
    ssa_h = P_h @ ZTU_h           (2048, 64)
    mssa = concat_h(ssa_h) @ W    (2048, 1024)
    return (mssa, mssa)

Sharding: 8 cores = 2 batches x 4 head-groups (4 heads each). Each core
computes its 4 heads end-to-end and a partial mssa (sum over its heads);
host adds the 4 partials per batch.

Key design points:
  - S is symmetric, so the exp'd score row-tiles F_j = F[block j, :] serve
    both as "query rows" (row sums -> softmax denominators) and as "key
    rows" (rhs of the P @ V matmul in the ssa^T orientation). No transposes
    of the big attention matrix are needed.
  - softmax is shift-invariant and exp() cannot overflow fp32 at these
    magnitudes, so no max pass: P = F / rowsum(F). The 1/rowsum (per query)
    is applied to ssa^T via a gpsimd partition_broadcast tile.
  - bf16 matmul inputs everywhere (fp32 matmul is 4x slower); fp32 PSUM.
  - head pairs are packed into the 128-wide PE array: scores row-packed
    (two K=64 at tile_position (0,0)/(64,0)); P@V column-packed (two M=64
    at (0,0)/(0,64)). PSUM accumulation groups are opened/closed by
    full-width rank-1 zero matmuls to keep one group per bank.
  - exp is split between ACT (exact, fused accum row sums) and DVE
    (16-bit Schraudolph: bits = round(A*(s/8)+B) written via int16 bitcast
    straight into the bf16 F tile; row sums via a x1.0 in-place
    tensor_scalar with accum_out). ~0.5% extra error, softmax cancels the
    constant-scale component.
  - pair 1's projection + V transposes are emitted inside pair 0's j-loop
    so they overlap the ACT/DVE-bound attention steady state instead of
    extending the serial startup.
"""

import numpy as np
import ml_dtypes

import concourse.bass as bass
from concourse import bacc
import concourse.mybir as mybir
import concourse.tile as tile
from concourse.masks import make_identity

BF16 = mybir.dt.bfloat16
F32 = mybir.dt.float32

B = 2
N = 2048
DIM = 1024
H = 16
DH = 64
SCALE = DH**-0.5
# 16-bit Schraudolph exp: bf16(bits), bits = round(A16S*s + B16), s = raw score
A16S = (2.0**7 / float(np.log(2.0))) * SCALE
B16 = 127.0 * 128.0 - 5.0 + 0.5

NCORES = 8
HPC = 4
RPC = HPC * DH  # 256 W rows per core
NB = N // 128  # 16 token blocks
KB = DIM // 128  # 8 contraction blocks

_CACHED = {}


def build_nc():
    nc = bacc.Bacc("TRN2", target_bir_lowering=False, debug=False)

    ztt_d = nc.dram_tensor("ztt", [DIM, N], BF16, kind="ExternalInput")
    wct_d = nc.dram_tensor("wct", [DIM, RPC], BF16, kind="ExternalInput")
    wc_d = nc.dram_tensor("wc", [RPC, DIM], BF16, kind="ExternalInput")
    out_d = nc.dram_tensor("out", [N, DIM], BF16, kind="ExternalOutput")

    with tile.TileContext(nc) as tc:
        with (
            tc.tile_pool(name="big", bufs=1) as big,
            tc.tile_pool(name="fpool", bufs=3) as fpool,
            tc.tile_pool(name="small", bufs=2) as small,
            tc.tile_pool(name="outp", bufs=3) as outp,
            tc.tile_pool(name="sps", bufs=2, space="PSUM") as sps,
            tc.tile_pool(name="pvps", bufs=4, space="PSUM") as pvps,
        ):
            ZTT = big.tile([128, KB * N], BF16, tag="ztt", name="ZTT")
            WCT = big.tile([128, KB * RPC], BF16, tag="wct", name="WCT")
            WC = big.tile([128, 2 * DIM], BF16, tag="wc", name="WC")
            ZTUT = big.tile([128, 2 * N], BF16, tag="ztut", name="ZTUT")
            V = big.tile([128, NB * RPC], BF16, tag="v", name="V")
            SSAT = big.tile([128, 2 * N], BF16, tag="ssat", name="SSAT")
            ID16 = big.tile([128, 128], BF16, tag="id16", name="ID16")
            ID32 = big.tile([128, 128], F32, tag="id32", name="ID32")
            ZEROS = big.tile([1, 512], BF16, tag="zeros", name="ZEROS")

            make_identity(nc, ID16)
            make_identity(nc, ID32)
            nc.vector.memset(ZEROS, 0.0)

            for kb in range(KB):
                nc.gpsimd.dma_start(
                    out=ZTT[:, kb * N : (kb + 1) * N],
                    in_=ztt_d[kb * 128 : (kb + 1) * 128, :],
                )
                nc.sync.dma_start(
                    out=WCT[:, kb * RPC : (kb + 1) * RPC],
                    in_=wct_d[kb * 128 : (kb + 1) * 128, :],
                )
            for p in range(2):
                nc.sync.dma_start(
                    out=WC[:, p * DIM : (p + 1) * DIM],
                    in_=wc_d[p * 128 : (p + 1) * 128, :],
                )

            # ---- deferred emission helpers -----------------------------
            def emit_proj_half(p, hq):
                """ZTUT[p, half hq] = (Wc_p @ ZT.T)[:, half] via 16 matmuls."""
                zps = sps.tile([128, 1024], F32, tag="s", name="zps")
                for kb in range(KB):
                    for cc in range(2):
                        nc.tensor.matmul(
                            zps[:, cc * 512 : (cc + 1) * 512],
                            WCT[:, kb * RPC + p * 128 : kb * RPC + (p + 1) * 128],
                            ZTT[
                                :,
                                kb * N + hq * 1024 + cc * 512 : kb * N
                                + hq * 1024
                                + (cc + 1) * 512,
                            ],
                            start=(kb == 0),
                            stop=(kb == KB - 1),
                        )
                nc.vector.tensor_copy(
                    ZTUT[:, p * N + hq * 1024 : p * N + (hq + 1) * 1024], zps
                )

            def emit_vtrans(p, tb):
                """V block tb gets pair p's 128 dims via PE transpose."""
                vps = sps.tile([128, 128], BF16, tag="s", name="vps")
                nc.tensor.transpose(
                    vps, ZTUT[:, p * N + tb * 128 : p * N + (tb + 1) * 128], ID16
                )
                nc.vector.tensor_copy(
                    V[:, tb * RPC + p * 128 : tb * RPC + (p + 1) * 128], vps
                )

            def emit_pair(p, deferred):
                """Attention j-loop for head pair p. `deferred` is a list of
                zero-arg emitters drained a little per j iteration (used to
                overlap pair 1's projection with pair 0's loop)."""
                pacc = [
                    small.tile([128, 2 * NB], F32, tag=f"pacc{a}", name=f"pacc{a}")
                    for a in range(2)
                ]
                pv = [pvps.tile([128, 512], F32, tag="pv", name="pv") for _ in range(4)]
                for cc in range(4):
                    nc.tensor.matmul(
                        pv[cc], ZEROS[0:1, 0:128], ZEROS[0:1, 0:512],
                        start=True, stop=False,
                    )

                for j in range(NB):
                    F = [
                        fpool.tile([128, N], BF16, tag=f"F{a}", name=f"F{a}")
                        for a in range(2)
                    ]
                    for hk in range(2):
                        stile = [
                            sps.tile([128, 1024], F32, tag="s", name="stile")
                            for _ in range(2)
                        ]
                        for a in range(2):
                            lhsT = ZTUT[
                                a * 64 : (a + 1) * 64,
                                p * N + j * 128 : p * N + (j + 1) * 128,
                            ]
                            for cc in range(2):
                                k0 = p * N + hk * 1024 + cc * 512
                                nc.tensor.matmul(
                                    stile[a][:, cc * 512 : (cc + 1) * 512],
                                    lhsT,
                                    ZTUT[a * 64 : (a + 1) * 64, k0 : k0 + 512],
                                    start=True,
                                    stop=True,
                                    tile_position=(a * 64, 0),
                                )
                        for a in range(2):
                            fslice = F[a][:, hk * 1024 : (hk + 1) * 1024]
                            acc = pacc[a][:, 2 * j + hk : 2 * j + hk + 1]
                            if not (a == 1 and j % 4 != 3):
                                nc.scalar.activation(
                                    fslice,
                                    stile[a],
                                    mybir.ActivationFunctionType.Exp,
                                    scale=SCALE,
                                    accum_out=acc,
                                )
                            else:
                                nc.vector.tensor_scalar(
                                    fslice.bitcast(mybir.dt.int16),
                                    stile[a],
                                    A16S,
                                    B16,
                                    mybir.AluOpType.mult,
                                    mybir.AluOpType.add,
                                )
                                nc.vector.tensor_scalar(
                                    fslice,
                                    fslice,
                                    1.0,
                                    0.0,
                                    mybir.AluOpType.mult,
                                    mybir.AluOpType.add,
                                    accum_out=acc,
                                )
                    for cc in range(4):
                        for a in range(2):
                            nc.tensor.matmul(
                                pv[cc][a * 64 : (a + 1) * 64, :],
                                V[
                                    :,
                                    j * RPC + p * 128 + a * 64 : j * RPC
                                    + p * 128
                                    + (a + 1) * 64,
                                ],
                                F[a][:, cc * 512 : (cc + 1) * 512],
                                start=False,
                                stop=False,
                                tile_position=(0, a * 64),
                            )
                    # drain a couple of deferred emissions per iteration
                    for _ in range(2):
                        if deferred:
                            deferred.pop(0)()
                for cc in range(4):
                    nc.tensor.matmul(
                        pv[cc], ZEROS[0:1, 0:128], ZEROS[0:1, 0:512],
                        start=False, stop=True,
                    )
                while deferred:
                    deferred.pop(0)()

                # ---- normalization: ssaT = pv * (1/rowsum[q]) ----------
                rec1p = []
                for a in range(2):
                    fs = small.tile([128, NB], F32, tag=f"fsum{a}", name=f"fsum{a}")
                    nc.vector.reduce_sum(
                        out=fs.rearrange("p (n one) -> p n one", one=1),
                        in_=pacc[a].rearrange("p (n two) -> p n two", two=2),
                        axis=mybir.AxisListType.X,
                    )
                    rr = small.tile([128, NB], F32, tag=f"rr{a}", name=f"rr{a}")
                    nc.vector.reciprocal(rr, fs)
                    rrt_ps = sps.tile([NB, 128], F32, tag="s", name="rrt_ps")
                    nc.tensor.transpose(rrt_ps, rr, ID32)
                    rrt = small.tile([NB, 128], F32, tag=f"rrt{a}", name=f"rrt{a}")
                    nc.vector.tensor_copy(rrt, rrt_ps)
                    r1 = small.tile([1, N], F32, tag=f"rec1p{a}", name=f"rec1p{a}")
                    nc.sync.dma_start(
                        out=r1.rearrange("one (n f) -> one n f", f=128),
                        in_=rrt.rearrange("n (one f) -> n one f", one=1),
                    )
                    rec1p.append(r1)
                recb = []
                for a in range(2):
                    rb = small.tile([64, N], F32, tag=f"recb{a}", name=f"recb{a}")
                    nc.gpsimd.partition_broadcast(rb, rec1p[a][0:1, :])
                    recb.append(rb)
                for cc in range(4):
                    for a in range(2):
                        nc.vector.tensor_mul(
                            SSAT[
                                a * 64 : (a + 1) * 64,
                                p * N + cc * 512 : p * N + (cc + 1) * 512,
                            ],
                            pv[cc][a * 64 : (a + 1) * 64, :],
                            recb[a][:, cc * 512 : (cc + 1) * 512],
                        )

            # ---- phase structure ---------------------------------------
            emit_proj_half(0, 0)
            emit_proj_half(0, 1)
            for tb in range(NB):
                emit_vtrans(0, tb)

            deferred = [
                lambda hq=hq: emit_proj_half(1, hq) for hq in range(2)
            ] + [lambda tb=tb: emit_vtrans(1, tb) for tb in range(NB)]
            emit_pair(0, deferred)
            emit_pair(1, [])

            # ---- final projection: out[q, e] = ssa @ Wc ----------------
            for qb in range(NB):
                fin = sps.tile([128, 1024], F32, tag="s", name="fin")
                for p in range(2):
                    lhsT = SSAT[:, p * N + qb * 128 : p * N + (qb + 1) * 128]
                    for ec in range(2):
                        nc.tensor.matmul(
                            fin[:, ec * 512 : (ec + 1) * 512],
                            lhsT,
                            WC[:, p * DIM + ec * 512 : p * DIM + (ec + 1) * 512],
                            start=(p == 0),
                            stop=(p == 1),
                        )
                osb = outp.tile([128, 1024], BF16, tag="osb", name="osb")
                if qb % 2 == 0:
                    nc.scalar.copy(osb, fin)
                else:
                    nc.vector.tensor_copy(osb, fin)
                eng = nc.sync if qb % 2 == 0 else nc.gpsimd
                eng.dma_start(out=out_d[qb * 128 : (qb + 1) * 128, :], in_=osb)

    nc.compile()
    return nc


def kernel(ZT, W):
    from concourse.bass_utils import run_bass_kernel_spmd

    ZT = np.asarray(ZT, dtype=np.float32)
    W = np.asarray(W, dtype=np.float32)
    if "nc" not in _CACHED:
        _CACHED["nc"] = build_nc()
    nc = _CACHED["nc"]

    bf = ml_dtypes.bfloat16
    in_maps = []
    for c in range(NCORES):
        b, g = c // 4, c % 4
        rows = slice(g * RPC, (g + 1) * RPC)
        in_maps.append(
            {
                "ztt": np.ascontiguousarray(ZT[b].T).astype(bf),
                "wct": np.ascontiguousarray(W[rows, :].T).astype(bf),
                "wc": np.ascontiguousarray(W[rows, :]).astype(bf),
            }
        )

    res = run_bass_kernel_spmd(nc, in_maps, list(range(NCORES)))
    outs = [np.asarray(res.results[c]["out"], dtype=np.float32) for c in range(NCORES)]
    mssa = np.stack(
        [outs[0] + outs[1] + outs[2] + outs[3], outs[4] + outs[5] + outs[6] + outs[7]],
        axis=0,
    )
    return (mssa, mssa)


# revision 18
# speedup vs baseline: 1.0072x; 1.0072x over previous
"""Trainium2 Bass kernel for nn_Attention_Encode (dense transformer block).

Reference computation (per batch b):
    ZTU  = ZT[b] @ W.T            (2048, 1024) -> heads (16, 2048, 64)
    S_h  = ZTU_h @ ZTU_h.T * s    (2048, 2048)   symmetric! (Q == K)
    P_h  = softmax(S_h)
    ssa_h = P_h @ ZTU_h           (2048, 64)
    mssa = concat_h(ssa_h) @ W    (2048, 1024)
    return (mssa, mssa)

Sharding: 8 cores = 2 batches x 4 head-groups (4 heads each). Each core
computes its 4 heads end-to-end and a partial mssa (sum over its heads);
host adds the 4 partials per batch.

Key design points:
  - S is symmetric, so the exp'd score row-tiles F_j = F[block j, :] serve
    both as "query rows" (row sums -> softmax denominators) and as "key
    rows" (rhs of the P @ V matmul in the ssa^T orientation). No transposes
    of the big attention matrix are needed.
  - softmax is shift-invariant and exp() cannot overflow fp32 at these
    magnitudes, so no max pass: P = F / rowsum(F). The 1/rowsum (per query)
    is applied to ssa^T via a gpsimd partition_broadcast tile.
  - bf16 matmul inputs everywhere (fp32 matmul is 4x slower); fp32 PSUM.
  - head pairs are packed into the 128-wide PE array: scores row-packed
    (two K=64 at tile_position (0,0)/(64,0)); P@V column-packed (two M=64
    at (0,0)/(0,64)). PSUM accumulation groups are opened/closed by
    full-width rank-1 zero matmuls to keep one group per bank.
  - exp is split between ACT (exact, fused accum row sums) and DVE
    (16-bit Schraudolph: bits = round(A*(s/8)+B) written via int16 bitcast
    straight into the bf16 F tile; row sums via a x1.0 in-place
    tensor_scalar with accum_out). ~0.5% extra error, softmax cancels the
    constant-scale component.
  - pair 1's projection + V transposes are emitted inside pair 0's j-loop
    so they overlap the ACT/DVE-bound attention steady state instead of
    extending the serial startup.
"""

import numpy as np
import ml_dtypes

import concourse.bass as bass
from concourse import bacc
import concourse.mybir as mybir
import concourse.tile as tile
from concourse.masks import make_identity

BF16 = mybir.dt.bfloat16
F32 = mybir.dt.float32

B = 2
N = 2048
DIM = 1024
H = 16
DH = 64
SCALE = DH**-0.5
# 16-bit Schraudolph exp: bf16(bits), bits = round(A16S*s + B16), s = raw score
A16S = (2.0**7 / float(np.log(2.0))) * SCALE
B16 = 127.0 * 128.0 - 5.0 + 0.5

NCORES = 8
HPC = 4
RPC = HPC * DH  # 256 W rows per core
NB = N // 128  # 16 token blocks
KB = DIM // 128  # 8 contraction blocks

_CACHED = {}


def build_nc():
    nc = bacc.Bacc("TRN2", target_bir_lowering=False, debug=False)

    ztt_d = nc.dram_tensor("ztt", [DIM, N], BF16, kind="ExternalInput")
    wct_d = nc.dram_tensor("wct", [DIM, RPC], BF16, kind="ExternalInput")
    wc_d = nc.dram_tensor("wc", [RPC, DIM], BF16, kind="ExternalInput")
    out_d = nc.dram_tensor("out", [N, DIM], BF16, kind="ExternalOutput")

    with tile.TileContext(nc) as tc:
        with (
            tc.tile_pool(name="big", bufs=1) as big,
            tc.tile_pool(name="fpool", bufs=3) as fpool,
            tc.tile_pool(name="small", bufs=2) as small,
            tc.tile_pool(name="outp", bufs=3) as outp,
            tc.tile_pool(name="sps", bufs=2, space="PSUM") as sps,
            tc.tile_pool(name="pvps", bufs=4, space="PSUM") as pvps,
        ):
            ZTT = big.tile([128, KB * N], BF16, tag="ztt", name="ZTT")
            WCT = big.tile([128, KB * RPC], BF16, tag="wct", name="WCT")
            WC = big.tile([128, 2 * DIM], BF16, tag="wc", name="WC")
            ZTUT = big.tile([128, 2 * N], BF16, tag="ztut", name="ZTUT")
            V = big.tile([128, NB * RPC], BF16, tag="v", name="V")
            SSAT = big.tile([128, 2 * N], BF16, tag="ssat", name="SSAT")
            ID16 = big.tile([128, 128], BF16, tag="id16", name="ID16")
            ID32 = big.tile([128, 128], F32, tag="id32", name="ID32")
            ZEROS = big.tile([1, 512], BF16, tag="zeros", name="ZEROS")

            make_identity(nc, ID16)
            make_identity(nc, ID32)
            nc.vector.memset(ZEROS, 0.0)

            for kb in range(KB):
                nc.gpsimd.dma_start(
                    out=ZTT[:, kb * N : (kb + 1) * N],
                    in_=ztt_d[kb * 128 : (kb + 1) * 128, :],
                )
                nc.sync.dma_start(
                    out=WCT[:, kb * RPC : (kb + 1) * RPC],
                    in_=wct_d[kb * 128 : (kb + 1) * 128, :],
                )
            for p in range(2):
                nc.sync.dma_start(
                    out=WC[:, p * DIM : (p + 1) * DIM],
                    in_=wc_d[p * 128 : (p + 1) * 128, :],
                )

            # ---- deferred emission helpers -----------------------------
            def emit_proj_half(p, hq):
                """ZTUT[p, half hq] = (Wc_p @ ZT.T)[:, half] via 16 matmuls."""
                zps = sps.tile([128, 1024], F32, tag="s", name="zps")
                for kb in range(KB):
                    for cc in range(2):
                        nc.tensor.matmul(
                            zps[:, cc * 512 : (cc + 1) * 512],
                            WCT[:, kb * RPC + p * 128 : kb * RPC + (p + 1) * 128],
                            ZTT[
                                :,
                                kb * N + hq * 1024 + cc * 512 : kb * N
                                + hq * 1024
                                + (cc + 1) * 512,
                            ],
                            start=(kb == 0),
                            stop=(kb == KB - 1),
                        )
                nc.vector.tensor_copy(
                    ZTUT[:, p * N + hq * 1024 : p * N + (hq + 1) * 1024], zps
                )

            def emit_vtrans(p, tb):
                """V block tb gets pair p's 128 dims via PE transpose."""
                vps = sps.tile([128, 128], BF16, tag="s", name="vps")
                nc.tensor.transpose(
                    vps, ZTUT[:, p * N + tb * 128 : p * N + (tb + 1) * 128], ID16
                )
                nc.vector.tensor_copy(
                    V[:, tb * RPC + p * 128 : tb * RPC + (p + 1) * 128], vps
                )

            def emit_pair(p, deferred):
                """Attention j-loop for head pair p. `deferred` is a list of
                zero-arg emitters drained a little per j iteration (used to
                overlap pair 1's projection with pair 0's loop)."""
                pacc = [
                    small.tile([128, 2 * NB], F32, tag=f"pacc{a}", name=f"pacc{a}")
                    for a in range(2)
                ]
                pv = [pvps.tile([128, 512], F32, tag="pv", name="pv") for _ in range(4)]
                for cc in range(4):
                    nc.tensor.matmul(
                        pv[cc], ZEROS[0:1, 0:128], ZEROS[0:1, 0:512],
                        start=True, stop=False,
                    )

                for j in range(NB):
                    F = [
                        fpool.tile([128, N], BF16, tag=f"F{a}", name=f"F{a}")
                        for a in range(2)
                    ]
                    for hk in range(2):
                        stile = [
                            sps.tile([128, 1024], F32, tag="s", name="stile")
                            for _ in range(2)
                        ]
                        for a in range(2):
                            lhsT = ZTUT[
                                a * 64 : (a + 1) * 64,
                                p * N + j * 128 : p * N + (j + 1) * 128,
                            ]
                            for cc in range(2):
                                k0 = p * N + hk * 1024 + cc * 512
                                nc.tensor.matmul(
                                    stile[a][:, cc * 512 : (cc + 1) * 512],
                                    lhsT,
                                    ZTUT[a * 64 : (a + 1) * 64, k0 : k0 + 512],
                                    start=True,
                                    stop=True,
                                    tile_position=(a * 64, 0),
                                )
                        for a in range(2):
                            fslice = F[a][:, hk * 1024 : (hk + 1) * 1024]
                            acc = pacc[a][:, 2 * j + hk : 2 * j + hk + 1]
                            if not (a == 1 and j % 4 != 3):
                                nc.scalar.activation(
                                    fslice,
                                    stile[a],
                                    mybir.ActivationFunctionType.Exp,
                                    scale=SCALE,
                                    accum_out=acc,
                                )
                            else:
                                nc.vector.tensor_scalar(
                                    fslice.bitcast(mybir.dt.int16),
                                    stile[a],
                                    A16S,
                                    B16,
                                    mybir.AluOpType.mult,
                                    mybir.AluOpType.add,
                                )
                                nc.vector.tensor_scalar(
                                    fslice,
                                    fslice,
                                    1.0,
                                    0.0,
                                    mybir.AluOpType.mult,
                                    mybir.AluOpType.add,
                                    accum_out=acc,
                                )
                    for cc in range(4):
                        for a in range(2):
                            nc.tensor.matmul(
                                pv[cc][a * 64 : (a + 1) * 64, :],
                                V[
                                    :,
                                    j * RPC + p * 128 + a * 64 : j * RPC
                                    + p * 128
                                    + (a + 1) * 64,
                                ],
                                F[a][:, cc * 512 : (cc + 1) * 512],
                                start=False,
                                stop=False,
                                tile_position=(0, a * 64),
                            )
                    # drain a couple of deferred emissions per iteration
                    for _ in range(2):
                        if deferred:
                            deferred.pop(0)()
                for cc in range(4):
                    nc.tensor.matmul(
                        pv[cc], ZEROS[0:1, 0:128], ZEROS[0:1, 0:512],
                        start=False, stop=True,
                    )
                while deferred:
                    deferred.pop(0)()

                # ---- normalization: ssaT = pv * (1/rowsum[q]) ----------
                rec1p = []
                for a in range(2):
                    fs = small.tile([128, NB], F32, tag=f"fsum{a}", name=f"fsum{a}")
                    nc.vector.reduce_sum(
                        out=fs.rearrange("p (n one) -> p n one", one=1),
                        in_=pacc[a].rearrange("p (n two) -> p n two", two=2),
                        axis=mybir.AxisListType.X,
                    )
                    rr = small.tile([128, NB], F32, tag=f"rr{a}", name=f"rr{a}")
                    nc.vector.reciprocal(rr, fs)
                    rrt_ps = sps.tile([NB, 128], F32, tag="s", name="rrt_ps")
                    nc.tensor.transpose(rrt_ps, rr, ID32)
                    rrt = small.tile([NB, 128], F32, tag=f"rrt{a}", name=f"rrt{a}")
                    nc.vector.tensor_copy(rrt, rrt_ps)
                    r1 = small.tile([1, N], F32, tag=f"rec1p{a}", name=f"rec1p{a}")
                    nc.sync.dma_start(
                        out=r1.rearrange("one (n f) -> one n f", f=128),
                        in_=rrt.rearrange("n (one f) -> n one f", one=1),
                    )
                    rec1p.append(r1)
                recb = []
                for a in range(2):
                    rb = small.tile([64, N], F32, tag=f"recb{a}", name=f"recb{a}")
                    recb.append(rb)
                for cc in range(4):
                    for a in range(2):
                        nc.gpsimd.partition_broadcast(
                            recb[a][:, cc * 512 : (cc + 1) * 512],
                            rec1p[a][0:1, cc * 512 : (cc + 1) * 512],
                        )
                for cc in range(4):
                    for a in range(2):
                        nc.vector.tensor_mul(
                            SSAT[
                                a * 64 : (a + 1) * 64,
                                p * N + cc * 512 : p * N + (cc + 1) * 512,
                            ],
                            pv[cc][a * 64 : (a + 1) * 64, :],
                            recb[a][:, cc * 512 : (cc + 1) * 512],
                        )

            # ---- phase structure ---------------------------------------
            emit_proj_half(0, 0)
            emit_proj_half(0, 1)
            for tb in range(NB):
                emit_vtrans(0, tb)

            deferred = [
                lambda hq=hq: emit_proj_half(1, hq) for hq in range(2)
            ] + [lambda tb=tb: emit_vtrans(1, tb) for tb in range(NB)]
            emit_pair(0, deferred)
            emit_pair(1, [])

            # ---- final projection: out[q, e] = ssa @ Wc ----------------
            for qb in range(NB):
                fin = sps.tile([128, 1024], F32, tag="s", name="fin")
                for p in range(2):
                    lhsT = SSAT[:, p * N + qb * 128 : p * N + (qb + 1) * 128]
                    for ec in range(2):
                        nc.tensor.matmul(
                            fin[:, ec * 512 : (ec + 1) * 512],
                            lhsT,
                            WC[:, p * DIM + ec * 512 : p * DIM + (ec + 1) * 512],
                            start=(p == 0),
                            stop=(p == 1),
                        )
                osb = outp.tile([128, 1024], BF16, tag="osb", name="osb")
                if qb % 2 == 0:
                    nc.scalar.copy(osb, fin)
                else:
                    nc.vector.tensor_copy(osb, fin)
                eng = nc.sync if qb % 2 == 0 else nc.gpsimd
                eng.dma_start(out=out_d[qb * 128 : (qb + 1) * 128, :], in_=osb)

    nc.compile()
    return nc


def kernel(ZT, W):
    from concourse.bass_utils import run_bass_kernel_spmd

    ZT = np.asarray(ZT, dtype=np.float32)
    W = np.asarray(W, dtype=np.float32)
    if "nc" not in _CACHED:
        _CACHED["nc"] = build_nc()
    nc = _CACHED["nc"]

    bf = ml_dtypes.bfloat16
    in_maps = []
    for c in range(NCORES):
        b, g = c // 4, c % 4
        rows = slice(g * RPC, (g + 1) * RPC)
        in_maps.append(
            {
                "ztt": np.ascontiguousarray(ZT[b].T).astype(bf),
                "wct": np.ascontiguousarray(W[rows, :].T).astype(bf),
                "wc": np.ascontiguousarray(W[rows, :]).astype(bf),
            }
        )

    res = run_bass_kernel_spmd(nc, in_maps, list(range(NCORES)))
    outs = [np.asarray(res.results[c]["out"], dtype=np.float32) for c in range(NCORES)]
    mssa = np.stack(
        [outs[0] + outs[1] + outs[2] + outs[3], outs[4] + outs[5] + outs[6] + outs[7]],
        axis=0,
    )
    return (mssa, mssa)
